# revision 9
# baseline (speedup 1.0000x reference)
"""Trainium2 Bass kernel: 3x3 conv2d (stride 1, pad 1), NCHW.

x (32, 64, 112, 112) f32, weight (1, 128, 64, 3, 3) f32 -> out (32, 128, 112, 112) f32.

Strategy: data-parallel over batch across 8 cores (4 images/core).
Per core, conv is computed as 9 PSUM-accumulating matmuls (one per kernel
tap): x is host-padded to (114, 114) so each tap's shifted input window is a
constant free-dim offset into the flat [in_c=64, 114*114] SBUF image. Output
is produced in padded row-major (112 x 114) layout and sliced on the host.
"""

import numpy as np

import concourse.bacc as bacc
import concourse.tile as tile
from concourse import mybir
from concourse.bass_utils import run_bass_kernel_spmd

# Problem constants (hardcoded per harness contract).
B, C, H, W = 32, 64, 112, 112
OC, KH, KW = 128, 3, 3
NCORES = 8
BPC = B // NCORES          # images per core
HP, WP = H + 2, W + 2      # host-padded input height/width (114)
XFLAT = HP * WP            # 12996 flat padded-input elements per channel
OFLAT = H * WP             # 12768 flat padded-output elements per channel
BLK = 512                  # matmul free-dim block (= 1 PSUM bank of fp32)
NBLK = (OFLAT + BLK - 1) // BLK  # 25 blocks (24 full + 1 of 480)

# matmul dtype: float32r streams fp32 through the PE at 1 cycle/row for
# free-dim >= 256 (vs 4 cycles/row for plain float32).
MM_DTYPE = mybir.dt.float32r

_cache = {}


def _build(repeat=1):
    """Build + compile the per-core Bass program (cached per process).

    repeat>1 runs the whole per-core conv `repeat` times back-to-back inside
    one NEFF (idempotent) — used by test.py to measure steady-state device
    time net of dispatch overhead.
    """
    key = ("nc", repeat)
    if key in _cache:
        return _cache[key]

    nc = bacc.Bacc("TRN2", target_bir_lowering=False, debug=False)
    x_ap = nc.dram_tensor(
        "x", [BPC, C, HP, WP], MM_DTYPE, kind="ExternalInput"
    ).ap()
    w_ap = nc.dram_tensor(
        "w", [C, KH * KW * OC], MM_DTYPE, kind="ExternalInput"
    ).ap()
    out_ap = nc.dram_tensor(
        "out", [BPC, OC, H, WP], mybir.dt.float32, kind="ExternalOutput"
    ).ap()

    with tile.TileContext(nc) as tc:
        with (
            tc.tile_pool(name="xpool", bufs=2) as xpool,
            tc.tile_pool(name="wpool", bufs=1) as wpool,
            tc.tile_pool(name="opool", bufs=4) as opool,
            tc.tile_pool(name="psum", bufs=4, space="PSUM") as pspool,
        ):
            wt = wpool.tile([C, KH * KW * OC], MM_DTYPE)
            nc.sync.dma_start(wt[:], w_ap[:])

            def conv_pass():
                for im in range(BPC):
                    xt = xpool.tile([C, XFLAT + 4], MM_DTYPE)
                    x_im = x_ap[im].rearrange("c h w -> c (h w)")
                    # Split the image load across partition halves for DMA
                    # port parallelism.
                    nc.sync.dma_start(xt[: C // 2, :XFLAT], x_im[: C // 2])
                    nc.sync.dma_start(xt[C // 2 :, :XFLAT], x_im[C // 2 :])
                    o_im = out_ap[im].rearrange("o h w -> o (h w)")

                    for blk in range(NBLK):
                        j0 = blk * BLK
                        n = min(BLK, OFLAT - j0)
                        ps = pspool.tile([OC, BLK], mybir.dt.float32)
                        for t in range(KH * KW):
                            dh, dw = divmod(t, KW)
                            off = j0 + dh * WP + dw
                            nc.tensor.matmul(
                                ps[:, :n],
                                lhsT=wt[:, t * OC : (t + 1) * OC],
                                rhs=xt[:, off : off + n],
                                start=(t == 0),
                                stop=(t == KH * KW - 1),
                            )
                        ot = opool.tile([OC, BLK], mybir.dt.float32)
                        nc.scalar.copy(ot[:, :n], ps[:, :n])
                        nc.sync.dma_start(o_im[:, j0 : j0 + n], ot[:, :n])

            if repeat == 1:
                conv_pass()
            else:
                with tc.For_i(0, repeat, 1):
                    conv_pass()

    nc.compile()
    _cache[key] = nc
    return nc


def run_on_device(nc, in_maps):
    """Single-exec jitted runner with device-resident inputs; returns a
    callable for repeated timing plus the output fetcher."""
    from jax.sharding import Mesh, NamedSharding, PartitionSpec
    from jax.experimental.shard_map import shard_map
    import jax

    from concourse.bass2jax import (
        _bass_exec_p,
        install_neuronx_cc_hook,
        partition_id_tensor,
    )

    install_neuronx_cc_hook()

    partition_name = nc.partition_id_tensor.name if nc.partition_id_tensor else None
    in_names, out_names, out_avals = [], [], []
    for alloc in nc.m.functions[0].allocations:
        if not isinstance(alloc, mybir.MemoryLocationSet):
            continue
        name = alloc.memorylocations[0].name
        if alloc.kind == "ExternalInput":
            if name != partition_name:
                in_names.append(name)
        elif alloc.kind == "ExternalOutput":
            out_names.append(name)
            out_avals.append(
                jax.core.ShapedArray(
                    tuple(alloc.tensor_shape), mybir.dt.np(alloc.dtype)
                )
            )
    n_params = len(in_names)
    all_in_names = list(in_names) + list(out_names)
    if partition_name is not None:
        all_in_names.append(partition_name)
    all_in_names = tuple(all_in_names)

    def body(*args):
        operands = list(args)
        if partition_name is not None:
            operands.append(partition_id_tensor())
        return tuple(
            _bass_exec_p.bind(
                *operands,
                out_avals=tuple(out_avals),
                in_names=all_in_names,
                out_names=tuple(out_names),
                lowering_input_output_aliases=(),
                sim_require_finite=True,
                sim_require_nnan=True,
                nc=nc,
            )
        )

    n_cores = len(in_maps)
    devices = jax.devices()[:n_cores]
    mesh = Mesh(np.asarray(devices), ("core",))
    nspecs = n_params + len(out_names)
    sharded = jax.jit(
        shard_map(
            body,
            mesh=mesh,
            in_specs=(PartitionSpec("core"),) * nspecs,
            out_specs=(PartitionSpec("core"),) * len(out_names),
            check_rep=False,
        )
    )
    concat_in = [
        np.concatenate([np.asarray(in_maps[c][nm]) for c in range(n_cores)], axis=0)
        for nm in in_names
    ]
    concat_zeros = [
        np.zeros((n_cores * a.shape[0], *a.shape[1:]), a.dtype) for a in out_avals
    ]
    sharding = NamedSharding(mesh, PartitionSpec("core"))
    dev_in = [jax.device_put(a, sharding) for a in concat_in]
    dev_zeros = [jax.device_put(a, sharding) for a in concat_zeros]

    def run():
        return sharded(*dev_in, *dev_zeros)

    return run, out_names, out_avals


def _prep_inputs(x, weight):
    """Host-side shard + layout prep. Returns per-core input maps."""
    xp = np.zeros((B, C, HP, WP), dtype=np.float32)
    xp[:, :, 1 : H + 1, 1 : W + 1] = x
    # lhsT layout: [in_c, tap*128 + out_c]
    w_prep = np.ascontiguousarray(
        weight[0].transpose(1, 2, 3, 0).reshape(C, KH * KW * OC)
    ).astype(np.float32)
    return [
        {"x": xp[c * BPC : (c + 1) * BPC], "w": w_prep} for c in range(NCORES)
    ]


def kernel(x, weight):
    x = np.asarray(x, dtype=np.float32)
    weight = np.asarray(weight, dtype=np.float32)
    nc = _build()
    in_maps = _prep_inputs(x, weight)
    res = run_bass_kernel_spmd(nc, in_maps, list(range(NCORES)))
    out = np.concatenate([res.results[c]["out"] for c in range(NCORES)], axis=0)
    return np.ascontiguousarray(out[:, :, :, :W])
